# revision 59
# baseline (speedup 1.0000x reference)
"""Trainium2 Bass kernel for nn_Classifier (segment_reduce).

Computation (reference semantics):
  attn  = concat(emb, pos) @ W_attn + b_attn          (S, T, 1)
  w     = softmax(attn, axis=1)                        per-segment over T
  segv  = sum_t w * emb                                (S, BERT)
  vecs  = segment_sum(segv, segment_ids, 64)           (64, BERT)
  out   = sigmoid(lrelu(lrelu(vecs@W1+b1)@W2+b2)@W3+b3)

Sharding: data-parallel over S across 8 NeuronCores (32 segments each),
AllReduce of the comment partials (bf16), replicated MLP.

Structure (bf16 end-to-end):
 - b_attn shifts all logits of a segment equally -> softmax-invariant -> dropped.
 - Host packs emb+pos per core into bf16, partition-major layout
   [128, sl, nt, 897] with columns [emb 768 | 1.0 | pos 128] so that
   (a) each 2-segment DMA is 128 contiguous 14.3KB descriptors (the
       first two DMAs carry 1 segment each so compute starts early),
   (b) the pooling matmul over columns [512:769] accumulates the ones
       column -> softmax denominator lands in the pooled row for free.
   W_attn is zero-padded at the ones column so logits are unaffected.
 - Logits split across two engines (both run ~1x for free-dim reduces,
   so neither paces the loop alone): half the token tiles use the DVE
   fused STT mul+reduce; the other half use a DVE tensor_tensor product
   (2x bf16 mode) followed by ACT Copy+accum_out for the reduce.
 - exp on scalar engine (bf16 out) -> pooling matmuls all-bf16
   (4x the fp32 PE stream rate).
 - Pooled rows [1, 769] drain via scalar copy + partition-scatter DMA
   into segvecs[32, 769] (the drain is emitted one segment late so the
   scalar stream never stalls the next exp).
 - Cross-core reduction: AllToAll with my segvecs block replicated into
   all 8 input slots == AllGather of the full [256, 769] segment array,
   but A2A always runs the one-shot Mesh algorithm (AR/AG pick RDH in
   this runtime, ~3x slower for this size). The segment->comment
   one-hot reduction (with 1/den folded in, k=128 x 2 halves) happens
   AFTER the gather on every core, so the pre-collective critical path
   is just the last drain + 8 replication DMAs.
 - MLP feature-major throughout (no transposes): bias via k=1 matmuls,
   LeakyReLU as one DVE tensor_scalar+tensor_tensor pair per layer,
   final layer emitted directly as [64, 6]
   (y2 chunks stationary), sigmoid as exp(-x) -> 1/(1+e) on DVE so the
   scalar engine never swaps activation tables after the initial Exp
   table load.
"""

import os
import sys

sys.path.insert(0, "/opt/trn_rl_repo")
# Ask the collectives runtime to avoid the RDH algorithm for the small
# gather (mesh has a much lower latency floor). Harmless if ignored.
os.environ.setdefault("NEURON_RT_DBG_RDH_CC", "0")

import ml_dtypes
import numpy as np

BF16 = ml_dtypes.bfloat16

# Full-problem dims (hardcoded per contract)
S, T, BERT, POS = 256, 512, 768, 128
FEAT = BERT + POS
FEAT2 = FEAT + 1  # [emb 768 | ones 1 | pos 128]
H1 = 1024
NCLS = 6
NCOM = 64
NCORES = 8
SEG_CHUNK = 2  # segments per input DMA

_CACHE = {}


def build_nc(n_cores, sl, t, bert, pos, h1, ncls, ncom):
    """Build the SPMD Bass program for one core (sl segments/core)."""
    import concourse.bass as bass
    import concourse.mybir as mybir
    import concourse.tile as tile
    from concourse import bacc

    f32 = mybir.dt.float32
    bf16 = mybir.dt.bfloat16
    AF = mybir.ActivationFunctionType
    OP = mybir.AluOpType

    feat2 = bert + pos + 1
    nt = t // 128          # token tiles per segment (4)
    nj1 = bert // 128      # k tiles layer1 (6)
    nj2 = h1 // 128        # k tiles layer2/3 (8)
    nh = h1 // 128         # output chunks of h1 (8)
    nchunks = sl // SEG_CHUNK
    xlen = nt * feat2      # free elems per segment (3588)

    nc = bacc.Bacc(
        "TRN2", target_bir_lowering=False, debug=False, num_devices=n_cores
    )

    epk_d = nc.dram_tensor("epk", [128, sl * xlen], bf16, kind="ExternalInput").ap()
    wab_d = nc.dram_tensor("wab", [128, feat2], bf16, kind="ExternalInput").ap()
    oneh_d = nc.dram_tensor("oneh", [128, 2, ncom], f32, kind="ExternalInput").ap()
    w1_d = nc.dram_tensor("w1", [128, nj1 * h1], bf16, kind="ExternalInput").ap()
    w2_d = nc.dram_tensor("w2", [128, nj2 * h1], bf16, kind="ExternalInput").ap()
    w3_d = nc.dram_tensor("w3", [128, nj2 * ncls], bf16, kind="ExternalInput").ap()
    b1_d = nc.dram_tensor("b1", [1, h1], bf16, kind="ExternalInput").ap()
    b2_d = nc.dram_tensor("b2", [1, h1], bf16, kind="ExternalInput").ap()
    b3_d = nc.dram_tensor("b3", [1, ncls], bf16, kind="ExternalInput").ap()
    out_d = nc.dram_tensor("out", [ncom, ncls], f32, kind="ExternalOutput").ap()

    # AllToAll with my block replicated into every input slot acts as an
    # AllGather, but A2A always runs the one-shot mesh algorithm (the RDH
    # path that makes AG/AR slow in this runtime doesn't apply to A2A).
    # We gather the per-segment pooled vectors (49KB/core) instead of the
    # one-hot-reduced comment partials (98KB): half the wire bytes, and the
    # segment->comment reduction happens after the gather as 12 matmuls.
    sl_all = n_cores * sl
    a2a_in_d = nc.dram_tensor("a2a_in", [sl_all, feat2 - pos], bf16).ap()
    a2a_out_d = nc.dram_tensor("a2a_out", [sl_all, feat2 - pos], bf16).ap()
    # tiny warmup collective (fires at kernel start, hidden under the
    # loop) so the collectives firmware is warm when the real A2A triggers
    warm_in_d = nc.dram_tensor("warm_in", [sl_all, 8], bf16).ap()
    warm_out_d = nc.dram_tensor("warm_out", [sl_all, 8], bf16).ap()

    epk_v = epk_d.rearrange("p (s x) -> p s x", x=xlen)

    with tile.TileContext(nc) as tc:
        with (
            tc.tile_pool(name="const", bufs=1) as const_pool,
            tc.tile_pool(name="ep", bufs=4) as ep_pool,
            tc.tile_pool(name="work", bufs=1) as work,
            tc.tile_pool(name="psv", bufs=3, space="PSUM") as psv,
            tc.tile_pool(name="pmisc", bufs=2, space="PSUM") as pmisc,
        ):
            # ---- constants ----
            wab_sb = const_pool.tile([128, feat2], bf16)
            nc.sync.dma_start(wab_sb, wab_d)
            oneh_sb = const_pool.tile([128, 2, ncom], f32)
            nc.sync.dma_start(oneh_sb, oneh_d)
            ones_bf = const_pool.tile([128, ncom], bf16)
            nc.gpsimd.memset(ones_bf, 1.0)

            # warmup collective: no data deps, content irrelevant
            nc.gpsimd.collective_compute(
                "AllToAll",
                OP.bypass,
                replica_groups=[list(range(n_cores))],
                ins=[warm_in_d],
                outs=[warm_out_d],
            )

            # ---- persistent working tiles ----
            L_sb = work.tile([128, nt * sl], f32)     # logits, col = s*nt + i
            E_sb = work.tile([128, nt * sl], bf16)    # exp(logits)
            prod = work.tile([128, feat2], bf16)      # DVE STT product scratch
            dump = work.tile([128, feat2], bf16)      # ACT accum-path out sink
            segvecs = work.tile([sl, feat2 - pos], bf16)  # [s, 768 segvec | den]

            # ---- MLP weight tiles (bf16 straight from HBM) ----
            w1b = const_pool.tile([128, nj1, h1], bf16)
            w2b = const_pool.tile([128, nj2, h1], bf16)
            w3b = const_pool.tile([128, nj2, ncls], bf16)
            b1b = const_pool.tile([1, h1], bf16)
            b2b = const_pool.tile([1, h1], bf16)
            b3b = const_pool.tile([1, ncls], bf16)

            # One weight-DMA chunk or small-tensor load per input chunk;
            # interleaved into the segment loop so the HBM ring stays
            # dense and the loads fully overlap compute.
            w1f = w1b.rearrange("p j h -> p (j h)")
            w2f = w2b.rearrange("p j h -> p (j h)")
            dma_jobs = [
                lambda: nc.sync.dma_start(w1f[:, 0 : nj1 * h1 // 2], w1_d[:, 0 : nj1 * h1 // 2]),
                lambda: nc.sync.dma_start(w1f[:, nj1 * h1 // 2 :], w1_d[:, nj1 * h1 // 2 :]),
                lambda: nc.sync.dma_start(w2f[:, 0 : nj2 * h1 // 2], w2_d[:, 0 : nj2 * h1 // 2]),
                lambda: nc.sync.dma_start(w2f[:, nj2 * h1 // 2 :], w2_d[:, nj2 * h1 // 2 :]),
            ]

            def _small_loads():
                nc.sync.dma_start(w3b.rearrange("p j c -> p (j c)"), w3_d)
                nc.sync.dma_start(b1b, b1_d)
                nc.sync.dma_start(b2b, b2_d)
                nc.sync.dma_start(b3b, b3_d)

            dma_jobs.append(_small_loads)

            # ---- main loop over local segments, SEG_CHUNK at a time ----
            sv_tiles = {}

            def drain_segvec(sp):
                # PSUM -> SBUF stage (scalar, cast bf16), then scatter DMA.
                stage = work.tile([1, feat2 - pos], bf16, tag="stage", bufs=3, name="stage")
                nc.scalar.copy(stage, sv_tiles.pop(sp))
                nc.sync.dma_start(segvecs[sp : sp + 1, :], stage)

            # First two DMAs carry one segment each so compute starts as
            # early as possible; steady state is SEG_CHUNK segments/DMA.
            sizes = [1, 1] + [SEG_CHUNK] * ((sl - 2) // SEG_CHUNK)
            assert sum(sizes) == sl
            s0 = 0
            for c, csz in enumerate(sizes):
                ep = ep_pool.tile([128, SEG_CHUNK, nt, feat2], bf16, tag="ep")
                nc.sync.dma_start(
                    ep.rearrange("p s i f -> p s (i f)")[:, 0:csz, :],
                    epk_v[:, s0 : s0 + csz, :],
                )
                if c >= 1 and dma_jobs:
                    dma_jobs.pop(0)()

                for sc in range(csz):
                    s = s0 + sc
                    # Attention logits, split across DVE and ACT so neither
                    # engine paces the loop alone (GPSIMD can't help: the
                    # Pool engine has no STT opcode and its tensor_reduce
                    # is partition-axis only):
                    #  - ACT-path tiles: DVE tensor_tensor product (bf16,
                    #    2x mode) then ACT Copy+accum_out does the reduce
                    #  - remaining tiles: DVE fused STT (1x, single op)
                    # measured balance: ACT ~112us vs DVE ~107us busy in the
                    # loop; shedding one ACT tile on 3 of 32 segments evens
                    # them out (each shift moves ~1.4us ACT -> ~0.5us DVE)
                    act_tiles = (3,) if s in (10, 20, 30) else (2, 3)
                    p2 = {}
                    for i in act_tiles:
                        p2[i] = work.tile(
                            [128, feat2], bf16, tag="p2", bufs=4, name="p2"
                        )
                        nc.vector.tensor_tensor(
                            p2[i], ep[:, sc, i, :], wab_sb, op=OP.mult
                        )
                    for i in range(nt):
                        if i in act_tiles:
                            continue
                        nc.vector.scalar_tensor_tensor(
                            prod,
                            ep[:, sc, i, :],
                            1.0,
                            wab_sb,
                            op0=OP.mult,
                            op1=OP.mult,
                            accum_out=L_sb[:, nt * s + i : nt * s + i + 1],
                        )
                    for i in act_tiles:
                        nc.scalar.activation(
                            dump,
                            p2[i],
                            AF.Copy,
                            accum_out=L_sb[:, nt * s + i : nt * s + i + 1],
                        )
                    # e = exp(logits), bf16 out for the pooling stationary
                    nc.scalar.activation(
                        E_sb[:, nt * s : nt * s + nt],
                        L_sb[:, nt * s : nt * s + nt],
                        AF.Exp,
                    )
                    # Drain the PREVIOUS segment's pooled row here so the
                    # scalar stream goes exp(s) -> copy(s-1): copy(s-1)'s
                    # wait (on matmuls(s-1)) is already satisfied, so
                    # exp(s+1) never stalls behind matmuls(s).
                    if s >= 1:
                        drain_segvec(s - 1)

                    # pooling: segvec[s] = E-weighted sum over tokens; the
                    # ones column at 768 accumulates the denominator.
                    sv = psv.tile([1, feat2 - pos], f32, tag="sv")
                    sv_tiles[s] = sv
                    for i in range(nt):
                        col = nt * s + i
                        for n0, n1 in ((0, 512), (512, feat2 - pos)):
                            nc.tensor.matmul(
                                sv[0:1, n0:n1],
                                E_sb[:, col : col + 1],
                                ep[:, sc, i, n0:n1],
                                start=(i == 0),
                                stop=(i == nt - 1),
                            )
                s0 += csz

            drain_segvec(sl - 1)
            while dma_jobs:
                dma_jobs.pop(0)()

            # ---- gather ALL pooled segment rows via AllToAll (mesh) ----
            # my [32, 769] block replicated into each rank's slot; the
            # output is the full, globally-ordered [256, 769] segment array
            a2a_in_v = a2a_in_d.rearrange("(r p) x -> r p x", p=sl)
            for r in range(n_cores):
                nc.sync.dma_start(a2a_in_v[r], segvecs)
            nc.gpsimd.collective_compute(
                "AllToAll",
                OP.bypass,
                replica_groups=[list(range(n_cores))],
                ins=[a2a_in_d],
                outs=[a2a_out_d],
            )
            segfull = work.tile([128, 2, feat2 - pos], bf16)
            nc.sync.dma_start(
                segfull, a2a_out_d.rearrange("(h p) x -> p h x", p=128)
            )

            # ---- one-hot reduce segments -> comments (k=128 halves) ----
            inv2 = work.tile([128, 2], f32)
            nc.vector.reciprocal(inv2, segfull[:, :, bert])
            oneh_sc2 = work.tile([128, 2, ncom], bf16)
            for h in range(2):
                nc.vector.tensor_scalar_mul(
                    oneh_sc2[:, h, :], oneh_sb[:, h, :], inv2[:, h : h + 1]
                )
            cmT = pmisc.tile([128, nj1, ncom], f32, tag="m")
            for j in range(nj1):
                for h in range(2):
                    nc.tensor.matmul(
                        cmT[:, j, :],
                        segfull[:, h, 128 * j : 128 * (j + 1)],
                        oneh_sc2[:, h, :],
                        start=(h == 0),
                        stop=(h == 1),
                    )
            vecsT = work.tile([128, nj1, ncom], bf16)
            nc.scalar.copy(
                vecsT.rearrange("p j c -> p (j c)"), cmT.rearrange("p j c -> p (j c)")
            )

            # ---- MLP, feature-major all the way (no transposes) ----
            def linearT(xT, njx, wb, brow, nchk):
                # hT[p, n, c] = sum_j wb[:, j, 128n+p]^T xT[:, j, c] + b
                hT = pmisc.tile([128, nchk, ncom], f32, tag="m", name="hT")
                for n in range(nchk):
                    for j in range(njx):
                        nc.tensor.matmul(
                            hT[:, n, :],
                            wb[:, j, 128 * n : 128 * (n + 1)],
                            xT[:, j, :],
                            start=(j == 0),
                            stop=False,
                        )
                    nc.tensor.matmul(
                        hT[:, n, :],
                        brow[0:1, 128 * n : 128 * (n + 1)],
                        ones_bf[0:1, 0:ncom],
                        start=False,
                        stop=True,
                    )
                # LeakyReLU on DVE: t = x*0.01 (PSUM->SBUF), y = max(t, x)
                hf = hT.rearrange("p n c -> p (n c)")
                tmp = work.tile([128, nchk * ncom], f32, tag="lr", bufs=2, name="lr")
                nc.vector.tensor_scalar_mul(tmp, hf, 0.01)
                yT = work.tile([128, nchk, ncom], bf16, tag="yT", bufs=2, name="yT")
                nc.vector.tensor_tensor(
                    yT.rearrange("p n c -> p (n c)"),
                    tmp,
                    hf,
                    op=OP.max,
                )
                return yT

            y1T = linearT(vecsT, nj1, w1b, b1b, nh)
            y2T = linearT(y1T, nj2, w2b, b2b, nh)

            # final layer directly as [com, cls]: y2 chunks stationary
            outP = pmisc.tile([ncom, ncls], f32, tag="m", name="outP")
            for j in range(nj2):
                nc.tensor.matmul(
                    outP,
                    y2T[:, j, :],
                    w3b[:, j, :],
                    start=(j == 0),
                    stop=False,
                )
            nc.tensor.matmul(
                outP,
                ones_bf[0:1, 0:ncom],
                b3b,
                start=False,
                stop=True,
            )
            # sigmoid(x) = 1 / (1 + exp(-x)) — Exp table already loaded
            en_sb = work.tile([ncom, ncls], f32)
            nc.scalar.activation(en_sb, outP, AF.Exp, scale=-1.0)
            p1_sb = work.tile([ncom, ncls], f32)
            nc.vector.tensor_scalar_add(p1_sb, en_sb, 1.0)
            out_sb = work.tile([ncom, ncls], f32)
            nc.vector.reciprocal(out_sb, p1_sb)
            nc.sync.dma_start(out_d, out_sb)

    nc.compile()
    return nc


def make_in_maps(
    embeddings,
    position_encodings,
    W_attn,
    W1,
    b1,
    W2,
    b2,
    W3,
    b3,
    segment_ids,
    n_cores,
    ncom,
):
    """Host-side sharding: pack bf16 partition-major inputs per core."""
    f32 = np.float32
    s_total = embeddings.shape[0]
    sl = s_total // n_cores
    bert = embeddings.shape[2]
    pos = position_encodings.shape[2]
    feat2 = bert + pos + 1
    nt = embeddings.shape[1] // 128
    nj1 = bert // 128
    h1 = W1.shape[1]
    nj2 = h1 // 128
    nh = h1 // 128
    ncls = W3.shape[1]

    # wab row: [W_e(768) | 0 | W_p(128)], tiled to 128 partitions
    wa = np.asarray(W_attn, dtype=f32).reshape(-1)
    row = np.concatenate([wa[:bert], [0.0], wa[bert:]]).astype(BF16)
    wab = np.ascontiguousarray(np.tile(row[None, :], (128, 1)))

    w1p = np.ascontiguousarray(
        np.asarray(W1, dtype=f32).astype(BF16).reshape(nj1, 128, h1).transpose(1, 0, 2)
    ).reshape(128, nj1 * h1)
    w2p = np.ascontiguousarray(
        np.asarray(W2, dtype=f32).astype(BF16).reshape(nj2, 128, h1).transpose(1, 0, 2)
    ).reshape(128, nj2 * h1)
    w3p = np.ascontiguousarray(
        np.asarray(W3, dtype=f32).astype(BF16).reshape(nj2, 128, ncls).transpose(1, 0, 2)
    ).reshape(128, nj2 * ncls)
    b1t = np.ascontiguousarray(np.asarray(b1, dtype=f32).reshape(1, -1).astype(BF16))
    b2t = np.ascontiguousarray(np.asarray(b2, dtype=f32).reshape(1, -1).astype(BF16))
    b3r = np.ascontiguousarray(np.asarray(b3, dtype=f32).reshape(1, -1).astype(BF16))

    seg = np.asarray(segment_ids).astype(np.int64).reshape(-1)
    common = {
        "wab": wab,
        "w1": w1p,
        "w2": w2p,
        "w3": w3p,
        "b1": b1t,
        "b2": b2t,
        "b3": b3r,
    }

    emb = np.asarray(embeddings, dtype=f32)
    posE = np.asarray(position_encodings, dtype=f32)

    # full one-hot (every core reduces the gathered full segment array),
    # packed [128, 2, ncom]: row (p, h) = segment h*128+p
    oneh_full = np.zeros((s_total, ncom), dtype=f32)
    oneh_full[np.arange(s_total), seg] = 1.0
    oneh_p = np.ascontiguousarray(
        oneh_full.reshape(2, 128, ncom).transpose(1, 0, 2)
    )

    in_maps = []
    for c in range(n_cores):
        # pack [128, sl, nt, feat2] with cols [emb | 1.0 | pos]
        arr = np.empty((128, sl, nt, feat2), dtype=BF16)
        e_c = emb[c * sl : (c + 1) * sl].reshape(sl, nt, 128, bert)
        p_c = posE[c * sl : (c + 1) * sl].reshape(sl, nt, 128, pos)
        arr[:, :, :, 0:bert] = e_c.astype(BF16).transpose(2, 0, 1, 3)
        arr[:, :, :, bert] = BF16(1.0)
        arr[:, :, :, bert + 1 :] = p_c.astype(BF16).transpose(2, 0, 1, 3)
        epk = np.ascontiguousarray(arr.reshape(128, sl * nt * feat2))

        in_maps.append({"epk": epk, "oneh": oneh_p, **common})
    return in_maps


def kernel(
    embeddings,
    position_encodings,
    W_attn,
    b_attn,
    W1,
    b1,
    W2,
    b2,
    W3,
    b3,
    segment_ids,
    num_comments,
):
    from concourse.bass_utils import run_bass_kernel_spmd

    assert int(num_comments) == NCOM
    assert embeddings.shape == (S, T, BERT)
    assert position_encodings.shape == (S, T, POS)
    # b_attn shifts every logit of a segment equally -> softmax-invariant.

    key = "full"
    if key not in _CACHE:
        _CACHE[key] = build_nc(NCORES, S // NCORES, T, BERT, POS, H1, NCLS, NCOM)
    nc = _CACHE[key]

    in_maps = make_in_maps(
        embeddings,
        position_encodings,
        W_attn,
        W1,
        b1,
        W2,
        b2,
        W3,
        b3,
        segment_ids,
        NCORES,
        NCOM,
    )
    res = run_bass_kernel_spmd(nc, in_maps, list(range(NCORES)))
    return np.asarray(res.results[0]["out"], dtype=np.float32)
